# revision 28
# baseline (speedup 1.0000x reference)
"""Causal MHA (GQA 16q/4kv, QK-RMSnorm, RoPE, tanh softcap 50) on 8 TRN2 cores.

Sharding: 8 shards = (batch b in {0,1}) x (kv-group g in {0..3}).
Each core handles one batch's one kv-head group: 4 Q heads + 1 KV head,
w_q/w_k/w_v column-sharded, w_o row-sharded; host sums the 4 partial
y outputs per batch.

Per-core dataflow, single fused loop over 16 q-chunks m (128 rows each):
  qkv[m] = x[m] @ wqkv (f32r), RMS-norm + RoPE (DVE/ACT),
  q/k transposed to [d, S] fp16 via PE, v -> fp16 SBUF.
  per head h: raw = qT.T @ kT (fp16, causal extent)
              t = tanh(raw/400) (ACT; = tanh(score/50), score = raw/8)
              p = exp(50 t) fp16 (ACT), diag-masked (DVE)
  one batched DMA-transpose of all 4 heads' p row -> pT chunks
  o_unnorm|denom = pT.T @ [v|1] (PE), o = o_unnorm * recip(denom)
  y[m] = o @ wo (fp16 PE) -> SBUF -> HBM
"""

import numpy as np

D_MODEL = 1024
SEQ = 2048
HD = 64
NQH = 4  # q heads per core
CAP = 50.0
EPS = 1e-5
THETA = 10000.0
P = 128
MC = SEQ // P  # 16 q-chunks
KT = D_MODEL // P  # 8 contraction chunks for projections
N_CORES = 8

_nc_cache = None


def _build_nc():
    import concourse.bass as bass
    import concourse.tile as tile
    from concourse import bacc, mybir
    from concourse.bass import ts
    from concourse.masks import make_identity

    F32 = mybir.dt.float32
    F32R = mybir.dt.float32r
    F16 = mybir.dt.float16
    AF = mybir.ActivationFunctionType
    ALU = mybir.AluOpType
    AX = mybir.AxisListType

    nc = bacc.Bacc("TRN2")
    xT_d = nc.declare_dram_parameter("xT", [D_MODEL, SEQ], F32R, isOutput=False)
    wqkv_d = nc.declare_dram_parameter("wqkv", [D_MODEL, 384], F32R, isOutput=False)
    wo_d = nc.declare_dram_parameter("wo", [256, D_MODEL], F16, isOutput=False)
    cs_d = nc.declare_dram_parameter("cs", [SEQ, 64], F32, isOutput=False)
    tri_d = nc.declare_dram_parameter("tri", [P, P], F16, isOutput=False)
    y_d = nc.declare_dram_parameter("y", [SEQ, D_MODEL], F32, isOutput=True)

    with tile.TileContext(nc) as tc:
        with (
            tc.tile_pool(name="singles", bufs=1) as singles,
            tc.tile_pool(name="xmp", bufs=3) as xmp,
            tc.tile_pool(name="ptmp", bufs=3) as ptmp,
            tc.tile_pool(name="small", bufs=4) as small,
            tc.tile_pool(name="tpool", bufs=2) as tpool,
            tc.tile_pool(name="ppool", bufs=2) as ppool,
            tc.tile_pool(name="ptp", bufs=2) as ptp,
            tc.tile_pool(name="opool", bufs=2) as opool,
            tc.tile_pool(name="otp", bufs=2) as otp,
            tc.tile_pool(name="psum_s", bufs=2, space="PSUM") as psum_s,
            tc.tile_pool(name="psum_tr", bufs=1, space="PSUM") as psum_tr,
            tc.tile_pool(name="psum_pv", bufs=1, space="PSUM") as psum_pv,
            tc.tile_pool(name="psum_pj", bufs=1, space="PSUM") as psum_pj,
            tc.tile_pool(name="psum_y", bufs=1, space="PSUM") as psum_y,
        ):
            idn32 = singles.tile([P, P], F32)
            make_identity(nc, idn32)
            idn16 = singles.tile([P, P], F16)
            make_identity(nc, idn16)
            tri_sb = singles.tile([P, P], F16)
            nc.scalar.dma_start(tri_sb, tri_d[:, :])
            magic_sb = singles.tile([P, 1], mybir.dt.int32)
            nc.vector.memset(magic_sb, 0x5F3759DF)
            wo_sb = singles.tile([P, 2, D_MODEL], F16)
            nc.scalar.dma_start(wo_sb, wo_d[:, :].rearrange("(o p) n -> p o n", p=P))
            wqkv_sb = singles.tile([P, KT, 384], F32R)
            nc.scalar.dma_start(
                wqkv_sb, wqkv_d[:, :].rearrange("(o p) n -> p o n", p=P)
            )
            cs_sb = singles.tile([P, MC, 64], F32)
            nc.scalar.dma_start(cs_sb, cs_d[:, :].rearrange("(t p) n -> p t n", p=P))
            v_sb = singles.tile([P, MC, 65], F16)
            nc.vector.memset(v_sb, 1.0)
            qT_sb = singles.tile([64, NQH, SEQ], F16)
            kT_sb = singles.tile([64, SEQ], F16)

            xT_r = xT_d[:, :].rearrange("(o p) s -> p o s", p=P)

            for m in range(MC):
                km = (m + 1) * P
                # ---- projections for chunk m ----
                xm = xmp.tile([P, KT, P], F32R, tag="xm")
                nc.scalar.dma_start(xm, xT_r[:, :, ts(m, P)])
                pj = psum_pj.tile([P, 384], F32, tag="pj", name="pj")
                for kt in range(KT):
                    nc.tensor.matmul(
                        pj,
                        lhsT=xm[:, kt, :],
                        rhs=wqkv_sb[:, kt, :],
                        start=(kt == 0),
                        stop=(kt == KT - 1),
                    )
                pjh = pj[:, 0:320].rearrange("p (h d) -> p h d", d=HD)
                sq = ptmp.tile([P, 5, HD], F32, tag="sq")
                nc.scalar.activation(sq, pjh, AF.Square)
                ssq = small.tile([P, 5], F32, tag="ssq")
                nc.vector.reduce_sum(ssq, sq, axis=AX.X)
                I32 = mybir.dt.int32
                ms = small.tile([P, 5], F32, tag="ms")
                nc.vector.tensor_scalar(ms, ssq, 1.0 / HD, EPS, ALU.mult, ALU.add)
                hbits = small.tile([P, 5], I32, tag="hbits")
                nc.vector.tensor_scalar(
                    hbits, ms.bitcast(I32), 1, None, ALU.logical_shift_right
                )
                y0 = small.tile([P, 5], F32, tag="y0")
                nc.vector.tensor_tensor(
                    y0.bitcast(I32),
                    magic_sb[:, :].to_broadcast((P, 5)),
                    hbits,
                    ALU.subtract,
                )
                rr = y0
                for _ in range(2):
                    u = small.tile([P, 5], F32, tag="u", name="u")
                    nc.vector.tensor_mul(u, rr, rr)
                    tnew = small.tile([P, 5], F32, tag="tnew", name="tnew")
                    nc.vector.tensor_mul(tnew, ms, u)
                    w = small.tile([P, 5], F32, tag="w", name="w")
                    nc.vector.tensor_scalar(w, tnew, -0.5, 1.5, ALU.mult, ALU.add)
                    rr2 = small.tile([P, 5], F32, tag="rr2", name="rr2")
                    nc.vector.tensor_mul(rr2, rr, w)
                    rr = rr2
                qh = ptmp.tile([P, 5, HD], F32, tag="qh")
                nc.vector.tensor_mul(qh, pjh, rr[:, :, None].to_broadcast((P, 5, HD)))
                # v (unnormalized, no rope): cols 320:384
                nc.vector.tensor_copy(v_sb[:, m, 0:64], pj[:, 320:384])
                # rope on the 5 q/k heads
                cosb = cs_sb[:, m, None, 0:32].to_broadcast((P, 5, 32))
                sinb = cs_sb[:, m, None, 32:64].to_broadcast((P, 5, 32))
                q1 = qh[:, :, 0:32]
                q2 = qh[:, :, 32:64]
                qr = ptmp.tile([P, 5, HD], F32, tag="qr")
                ta = ptmp.tile([P, 5, 32], F32, tag="ta")
                tb = ptmp.tile([P, 5, 32], F32, tag="tb")
                nc.vector.tensor_mul(ta, q1, cosb)
                nc.vector.tensor_mul(tb, q2, sinb)
                nc.vector.tensor_tensor(qr[:, :, 0:32], ta, tb, ALU.subtract)
                tc2 = ptmp.tile([P, 5, 32], F32, tag="tc2")
                td = ptmp.tile([P, 5, 32], F32, tag="td")
                nc.vector.tensor_mul(tc2, q2, cosb)
                nc.vector.tensor_mul(td, q1, sinb)
                nc.vector.tensor_tensor(qr[:, :, 32:64], tc2, td, ALU.add)
                # transposes into fp16 [d, S] layout
                for h in range(NQH):
                    tq = psum_tr.tile([P, P], F32, tag="tr")
                    nc.tensor.transpose(tq[0:64, :], qr[:, h, :], idn32)
                    nc.vector.tensor_copy(qT_sb[:, h, ts(m, P)], tq[0:64, :])
                tk = psum_tr.tile([P, P], F32, tag="tr")
                nc.tensor.transpose(tk[0:64, :], qr[:, 4, :], idn32)
                nc.vector.tensor_copy(kT_sb[:, ts(m, P)], tk[0:64, :])

                # ---- attention row m ----
                p_m = ppool.tile([P, NQH, km], F16, tag="p")
                for h in range(NQH):
                    lhsT = qT_sb[:, h, ts(m, P)]
                    t_h = tpool.tile([P, SEQ], F32, tag="t")
                    for base in range(0, km, 1024):
                        w_sub = min(1024, km - base)
                        pss = psum_s.tile([P, 1024], F32, tag="s")
                        for kb in range(0, w_sub, 512):
                            wb = min(512, w_sub - kb)
                            nc.tensor.matmul(
                                pss[:, kb : kb + wb],
                                lhsT=lhsT,
                                rhs=kT_sb[:, base + kb : base + kb + wb],
                                start=True,
                                stop=True,
                            )
                        nc.scalar.activation(
                            t_h[:, base : base + w_sub],
                            pss[:, 0:w_sub],
                            AF.Tanh,
                            scale=1.0 / (8.0 * CAP),
                        )
                    nc.scalar.activation(
                        p_m[:, h, :], t_h[:, 0:km], AF.Exp, scale=CAP
                    )
                    # causal mask on the diagonal chunk
                    nc.vector.tensor_mul(
                        p_m[:, h, km - P : km], p_m[:, h, km - P : km], tri_sb
                    )
                # batched transpose per head
                pT = ptp.tile([P, NQH * MC, P], F16, tag="pT")
                for h in range(NQH):
                    nc.sync.dma_start_transpose(
                        pT[:, h * (m + 1) : (h + 1) * (m + 1), :],
                        p_m[:, h, :],
                    )
                o_sb = opool.tile([P, NQH, HD], F16, tag="o")
                for h in range(NQH):
                    pv = psum_pv.tile([P, 65], F32, tag="pv")
                    for kc in range(m + 1):
                        nc.tensor.matmul(
                            pv,
                            lhsT=pT[:, h * (m + 1) + kc, :],
                            rhs=v_sb[:, kc, :],
                            start=(kc == 0),
                            stop=(kc == m),
                        )
                    rc = small.tile([P, 1], F32, tag="rc")
                    nc.vector.reciprocal(rc, pv[:, 64:65])
                    nc.vector.tensor_scalar_mul(o_sb[:, h, :], pv[:, 0:64], rc)
                oT = otp.tile([P, 2, P], F16, tag="oT")
                for g in range(2):
                    to = psum_tr.tile([P, P], F16, tag="tr")
                    nc.tensor.transpose(to, o_sb[:, 2 * g : 2 * g + 2, :], idn16)
                    nc.vector.tensor_copy(oT[:, g, :], to)
                y_sb = opool.tile([P, D_MODEL], F32, tag="ysb")
                for nh in range(2):
                    yp = psum_y.tile([P, 512], F32, tag="y")
                    for g in range(2):
                        nc.tensor.matmul(
                            yp,
                            lhsT=oT[:, g, :],
                            rhs=wo_sb[:, g, ts(nh, 512)],
                            start=(g == 0),
                            stop=(g == 1),
                        )
                    nc.vector.tensor_copy(y_sb[:, ts(nh, 512)], yp)
                nc.scalar.dma_start(y_d[ts(m, P), :], y_sb)
    nc.finalize()
    return nc


def get_nc():
    global _nc_cache
    if _nc_cache is None:
        _nc_cache = _build_nc()
    return _nc_cache


def make_in_maps(x, w_q, w_k, w_v, w_o):
    x = np.asarray(x, np.float32)
    w_q = np.asarray(w_q, np.float32)
    w_k = np.asarray(w_k, np.float32)
    w_v = np.asarray(w_v, np.float32)
    w_o = np.asarray(w_o, np.float32)

    inv_freq = 1.0 / (THETA ** (np.arange(0, HD, 2, dtype=np.float32) / HD))
    freqs = np.arange(SEQ, dtype=np.float32)[:, None] * inv_freq[None, :]
    cs = np.concatenate(
        [np.cos(freqs), np.sin(freqs)], axis=1
    ).astype(np.float32)  # (S, 64)
    tri = np.tril(np.ones((P, P), np.float16))

    in_maps = []
    for c in range(N_CORES):
        b, g = divmod(c, 4)
        wqkv = np.concatenate(
            [
                w_q[:, g * 256 : (g + 1) * 256],
                w_k[:, g * 64 : (g + 1) * 64],
                w_v[:, g * 64 : (g + 1) * 64],
            ],
            axis=1,
        ).astype(np.float32)
        in_maps.append(
            {
                "xT": np.ascontiguousarray(x[b].T),
                "wqkv": np.ascontiguousarray(wqkv),
                "wo": np.ascontiguousarray(
                    w_o[g * 256 : (g + 1) * 256, :]
                ).astype(np.float16),
                "cs": cs,
                "tri": tri,
            }
        )
    return in_maps


def kernel(x, w_q, w_k, w_v, w_o):
    from concourse.bass_utils import run_bass_kernel_spmd

    nc = get_nc()
    in_maps = make_in_maps(x, w_q, w_k, w_v, w_o)
    res = run_bass_kernel_spmd(nc, in_maps, list(range(N_CORES))).results
    y = np.zeros((2, SEQ, D_MODEL), np.float32)
    for c in range(N_CORES):
        y[c // 4] += res[c]["y"]
    return y
